# revision 57
# baseline (speedup 1.0000x reference)
"""Trainium2 Bass kernel for nn_Caps2dMatwo (capsule conv + matwo dual routing).

Final (v25): v2 m-layout (cp, a, b, tp; cp forced outer by the conv-channel
coupling), chunk schedule [5,6,9,9,7], phase-shifted software pipeline
(ph0 of chunk c+1 issued before ph4 of chunk c, double-buffered scratch),
patch-DMA prefetch one block ahead, and the ENTIRE routing chain (coord,
sums, trees, routstep, stats side-chain, sigmoid) on the Vector engine --
on this FIFO-queued machine, any cross-engine hop in the serial chain
stalls everything behind it, so consolidation beat load-balancing in four
separate measured steps. ACT runs only table-bound activations (Square/
Ln/Exp) and PSUM drains; GpSimd runs nothing in consume. bf16 sigmoid/
bacc intermediates; reciprocal_approx_fast for the squash denominator.

Sharding: 8 cores = (batch n: 4) x (h-half: 2); each core computes a 48-row
slab of one batch element independently (halo via host padding).

Layouts (per core):
  patches P [96, T0, 36blk, 4j, 32z]  (host im2col; blocks of 128 px)
  conv psum X [(j,z)=128, (s2, pix32, co8)]
  transform m-index = 32*cp + 8*a + 2*b + tp  (t = 2cp+tp, z_out = 4a+b)
  raw m = 8*cp + 2*a + tp
  U (pixel-major) [128px, 9s, 4i, 2pa, 128m]; raw [128px, 9s, 4i, 32]
  routing all on-chip; v3 [128px, 9s, 2pa, 128m] -> DRAM bf16, host unscramble.
"""
import sys
import numpy as np

sys.path.insert(0, "/opt/trn_rl_repo")

import concourse.bass as bass
import concourse.bacc as bacc
import concourse.mybir as mybir
from concourse import tile
from concourse.bass_utils import run_bass_kernel_spmd
import ml_dtypes

BF16 = mybir.dt.float16
F32 = mybir.dt.float32
AL = mybir.AluOpType
AF = mybir.ActivationFunctionType

T0, T1, Z, H, W, HC = 4, 8, 32, 96, 96, 48
NBLK = 36
LN_EIGHTH = float(np.log(0.125))
CS = [5, 6, 9, 9, 7]                  # chunk sizes (s-blocks of 128 px)
S0S = [0, 5, 11, 20, 29]              # chunk offsets


# ----------------------------------------------------------------------------
# host-side weight/layout construction
# ----------------------------------------------------------------------------

def _build_weights(W_conv, W_pos, W_app, b_app):
    CW = np.zeros((96, T0, 32, 8), np.float32)
    for hi in range(8):
        for wi in range(12):
            for pi in range(4):
                for pj in range(8):
                    dy, dx = hi - pi, wi - pj
                    if 0 <= dy < 5 and 0 <= dx < 5:
                        CW[hi * 12 + wi, :, pi * 8 + pj, :] = W_conv[:, dy, dx, 0, :]

    m_pos = np.stack([W_pos[i].reshape(T1, 4, 4) for i in range(T0)])
    m_app = np.stack([W_app[i].reshape(T1, 4, 4) for i in range(T0)])
    nrm = np.sqrt(np.maximum((m_pos ** 2).sum(axis=2, keepdims=True), 1e-12))
    m_pos = m_pos / nrm

    # TW2[32j+z, i, pa, 32cp+8a+2b+tp] = M[i, 2cp+tp, c, b],  z = 16tp+4a+c
    TW2 = np.zeros((128, T0, 2, 128), np.float32)
    for i in range(T0):
        for pa, M in ((0, m_pos), (1, m_app)):
            blk = np.zeros((32, 128), np.float32)
            for cp in range(4):
                for tp in range(2):
                    t = 2 * cp + tp
                    for a in range(4):
                        for b in range(4):
                            m = 32 * cp + 8 * a + 2 * b + tp
                            for c in range(4):
                                blk[16 * tp + 4 * a + c, m] = M[i, t, c, b]
            for j in range(4):
                TW2[32 * j:32 * j + 32, i, pa] = blk

    # RW3[32j + 16tp+4a+3, cp, 8cp+2a+tp] = 1
    RW3 = np.zeros((128, 4, 32), np.float32)
    for cp in range(4):
        for tp in range(2):
            for a in range(4):
                for j in range(4):
                    RW3[32 * j + 16 * tp + 4 * a + 3, cp, 8 * cp + 2 * a + tp] = 1.0

    # KAB[32cp+8a+2b+tp, i] = b_app[i, 2cp+tp] * sum_c m_app[i, 2cp+tp, c, b]
    KAB = np.zeros((128, T0), np.float32)
    for i in range(T0):
        for cp in range(4):
            for tp in range(2):
                t = 2 * cp + tp
                for a in range(4):
                    for b in range(4):
                        m = 32 * cp + 8 * a + 2 * b + tp
                        KAB[m, i] = b_app[i, t] * m_app[i, t, :, b].sum()
    return CW, TW2, RW3, KAB


_PH = np.arange(NBLK) // 3
_B3 = np.arange(NBLK) % 3
_HIDX = (4 * _PH)[:, None] + np.arange(8)[None, :]
_PWJ = (4 * _B3)[:, None] + np.arange(4)[None, :]
_WIDX = (8 * _PWJ)[:, :, None] + np.arange(12)[None, None, :]


def _build_patches(pad):
    g = pad[:, :, _HIDX[:, None, :, None], _WIDX[:, :, None, :]]
    return np.ascontiguousarray(
        g.transpose(4, 5, 0, 2, 3, 1).reshape(96, T0, NBLK, 4, Z))


def _pixel_coords(hh):
    xs = np.zeros((128, NBLK, 2), np.float32)
    for b in range(NBLK):
        ph, b3 = b // 3, b % 3
        for j in range(4):
            for pi in range(4):
                for pj in range(8):
                    part = j * 32 + pi * 8 + pj
                    xs[part, b, 0] = (8 * (4 * b3 + j) + pj) / W
                    xs[part, b, 1] = (4 * ph + pi + 48 * hh) / H
    return xs


# ----------------------------------------------------------------------------
# device kernel
# ----------------------------------------------------------------------------

class _Pools:
    pass


def _alloc_chunk(P, tch):
    U_t = P.upool.tile([128, 9, T0, 2, 128], BF16, name=f"U{tch}", tag="ubig",
                       bufs=2)
    rawt = P.spool.tile([128, 9, T0, 32], BF16, name=f"rawt{tch}", tag="rawt",
                        bufs=2)
    return U_t, rawt


def _dma_pt(nc, P, tch, i, P_d):
    """Patch load for block (tch, i) — issued one block early so the conv
    never waits on HBM."""
    s0, ss = S0S[tch], CS[tch]
    pt = P.ppool.tile([96, 9, 4, Z], BF16, name="pt", tag="pt")
    nc.sync.dma_start(pt[:, 0:ss], P_d[:, i, s0:s0 + ss])
    return pt


def _produce_i(nc, P, tch, i, P_d, cw, tw, rw, kab, U_t, rawt, pt):
    """conv + transform + raw for one input-capsule block of chunk tch."""
    s0, ss = S0S[tch], CS[tch]
    xsb = P.xpool.tile([128, 9, 32, 8], BF16, name="xsb", tag="xsb")
    # conv: ss matmuls, drained in pairs (ACT)
    g = 0
    while g < ss:
        ns = min(2, ss - g)
        cps = P.pscv.tile([128, 2, 256], F32, name="cps", tag="cv")
        for s2 in range(ns):
            nc.tensor.matmul(
                cps[:, s2],
                pt[:, g + s2].rearrange("p j z -> p (j z)"),
                cw[:, i].rearrange("p f c -> p (f c)"),
                start=True, stop=True)
        nc.scalar.copy(
            xsb[:, g:g + ns].rearrange("p s f c -> p (s f c)"),
            cps[:, 0:ns].rearrange("p s f -> p (s f)"))
        g += ns
    # transform pos/app: per (pa, rp) psum tile [128, (ss, pix32)]
    for pa in range(2):
        stg = P.spool.tile([128, 9, 4, 32], BF16, name=f"stg{pa}",
                           tag=f"stg{pa}")
        for rp in range(4):
            ups = P.psuh.tile([128, 9, 32], F32, name="ups", tag="uh")
            for cp in range(4):
                nc.tensor.matmul(
                    ups[32 * cp:32 * cp + 32, 0:ss],
                    tw[32 * rp:32 * rp + 32, i, pa, 32 * cp:32 * cp + 32],
                    xsb[32 * rp:32 * rp + 32, 0:ss, :, pa * 4 + cp],
                    start=True, stop=True, tile_position=(32 * rp, 32 * cp))
            if pa == 0:
                nc.scalar.copy(stg[:, 0:ss, rp, :], ups[:, 0:ss])
            else:
                # drain + bias add (KAB is per-partition in m-space)
                nc.scalar.activation(stg[:, 0:ss, rp, :], ups[:, 0:ss],
                                     AF.Identity, bias=kab[:, i:i + 1])
        nc.sync.dma_start(
            U_t[:, 0:ss, i, pa, :],
            stg[:, 0:ss].rearrange("p s j f -> p (s j f)"), transpose=True)
    # raw extraction (pos conv channels, c=3 picks), accumulated over cp
    rstg = P.spool.tile([32, 9, 4, 32], BF16, name="rstg", tag="rstg")
    for rp in range(4):
        rps = P.psrw.tile([32, 9, 32], F32, name="rps", tag="rw")
        for cp in range(4):
            nc.tensor.matmul(
                rps[:, 0:ss].rearrange("p s x -> p (s x)"),
                rw[32 * rp:32 * rp + 32, cp],
                xsb[32 * rp:32 * rp + 32, 0:ss, :, cp],
                start=(cp == 0), stop=(cp == 3),
                tile_position=(32 * rp, 0))
        nc.scalar.copy(rstg[:, 0:ss, rp, :], rps[:, 0:ss])
    nc.sync.dma_start(
        rawt[:, 0:ss, i, :],
        rstg[:, 0:ss].rearrange("p s j f -> p (s j f)"), transpose=True)


def _tree4(nc, eng, P, src4d, G, op, pref, sz, out_dt=BF16):
    """src4d: AP [128, G, 4a, 4b, 2tp] -> [128, G, 2] tile (op-reduce over
    the middle 4x4) on engine `eng`. sz: tile-size class ('b' big/'s' small)."""
    m1 = P.mpool.tile([128, 288 if sz == 'b' else 36, 2, 4, 2], BF16,
                      name=f"{pref}m1", tag=f"{sz}m1")
    eng.tensor_tensor(m1[:, 0:G], src4d[:, :, 0:2], src4d[:, :, 2:4], op=op)
    m2 = P.mpool.tile([128, 288 if sz == 'b' else 36, 4, 2], BF16,
                      name=f"{pref}m2", tag=f"{sz}m2")
    eng.tensor_tensor(m2[:, 0:G], m1[:, 0:G, 0], m1[:, 0:G, 1], op=op)
    m3 = P.mpool.tile([128, 288 if sz == 'b' else 36, 2, 2], BF16,
                      name=f"{pref}m3", tag=f"{sz}m3")
    eng.tensor_tensor(m3[:, 0:G], m2[:, 0:G, 0:2], m2[:, 0:G, 2:4], op=op)
    m4 = P.mpool.tile([128, 288 if sz == 'b' else 36, 2], out_dt,
                      name=f"{pref}m4", tag=f"{sz}m4")
    eng.tensor_tensor(m4[:, 0:G], m3[:, 0:G, 0], m3[:, 0:G, 1], op=op)
    return m4


def _stats(nc, P, p, ss, iter1, tag, sfp, sfa):
    """p: tile [128, 9, 2, 128] (m = (cp,a,b,tp)). Writes per-(s,t) scale
    factors sfp/sfa [128, 9, 8] bf16 (valid [:, :ss]); pos tree on Vector,
    app tree on GpSimd."""
    G = ss * 4
    # pos: sfp = 1/max|p_pos| = exp(-0.5 ln(max p^2))  (scale-invariant)
    pab = P.mpool.tile([128, 9, 128], BF16, name=f"pab{tag}", tag="pab")
    nc.vector.tensor_tensor(pab[:, 0:ss], p[:, 0:ss, 0], p[:, 0:ss, 0],
                            op=AL.mult)
    pv = pab[:, 0:ss].rearrange("p s (cp a b tp) -> p (s cp) a b tp",
                                a=4, b=4, tp=2)
    mx = _tree4(nc, nc.vector, P, pv, G, AL.max, f"mx{tag}", 's', out_dt=F32)
    lnp = P.mpool.tile([128, 36, 2], F32, name=f"lnp{tag}", tag="lnp")
    nc.scalar.activation(lnp[:, 0:G].rearrange("p g tp -> p (g tp)"),
                         mx[:, 0:G].rearrange("p g tp -> p (g tp)"),
                         AF.Ln, bias=P.zerob[:, 0:1])
    nc.scalar.activation(
        sfp[:, 0:ss].rearrange("p s t -> p (s t)"),
        lnp[:, 0:G].rearrange("p g tp -> p (g tp)"),
        AF.Exp, scale=-0.5, bias=P.zerob[:, 0:1])
    # app: n2 = sum p_app^2 (x0.25 on iter1 via activation scale);
    # sfa = n2'/(1+n2') / sqrt(n2'+eps)  (x0.5 iter1 via exp bias)
    asq = P.mpool.tile([128, 9, 128], BF16, name=f"asq{tag}", tag="asq")
    nc.vector.tensor_tensor(asq[:, 0:ss], p[:, 0:ss, 1], p[:, 0:ss, 1],
                            op=AL.mult)
    av = asq[:, 0:ss].rearrange("p s (cp a b tp) -> p (s cp) a b tp",
                                a=4, b=4, tp=2)
    n2 = _tree4(nc, nc.vector, P, av, G, AL.add, f"n2{tag}", 's', out_dt=F32)
    scl = 0.25 if iter1 else 1.0
    g = P.mpool.tile([128, 36, 2], F32, name=f"g{tag}", tag="lga")
    nc.scalar.activation(g[:, 0:G].rearrange("p g tp -> p (g tp)"),
                         n2[:, 0:G].rearrange("p g tp -> p (g tp)"),
                         AF.Ln, bias=P.epsb[:, 0:1], scale=scl)
    h = P.mpool.tile([128, 36, 2], BF16, name=f"h{tag}", tag="h")
    nc.scalar.activation(h[:, 0:G].rearrange("p g tp -> p (g tp)"),
                         g[:, 0:G].rearrange("p g tp -> p (g tp)"),
                         AF.Exp, scale=-0.5,
                         bias=P.ln8b[:, 0:1] if iter1 else P.zerob[:, 0:1])
    den = P.mpool.tile([128, 36, 2], F32, name=f"den{tag}", tag="den")
    nc.vector.tensor_scalar(den[:, 0:G], n2[:, 0:G], scl, 1.0,
                            op0=AL.mult, op1=AL.add)
    rec = P.mpool.tile([128, 36, 2], F32, name=f"rec{tag}", tag="rec")
    nc.vector.reciprocal_approx_fast(rec[:, 0:G], den[:, 0:G])
    u1 = P.mpool.tile([128, 36, 2], F32, name=f"u1{tag}", tag="u1")
    nc.vector.tensor_tensor(u1[:, 0:G], n2[:, 0:G], rec[:, 0:G], op=AL.mult)
    nc.vector.tensor_tensor(
        sfa[:, 0:ss],
        u1[:, 0:G].rearrange("p (s cp) tp -> p s (cp tp)", cp=4),
        h[:, 0:G].rearrange("p (s cp) tp -> p s (cp tp)", cp=4), op=AL.mult)


def _sigmoid(nc, P, bacc_t, ss, tag):
    """r = 1/(1+exp(-b)) -> bf16 [128, 9, T0, 8] (valid [:, :ss])."""
    e = P.mpool.tile([128, 9, T0, 8], BF16, name=f"e{tag}", tag="sge")
    nc.scalar.activation(e[:, 0:ss].rearrange("p s i t -> p (s i t)"),
                         bacc_t[:, 0:ss].rearrange("p s i t -> p (s i t)"),
                         AF.Exp, scale=-1.0, bias=P.zerob[:, 0:1])
    nc.vector.tensor_scalar_add(e[:, 0:ss], e[:, 0:ss], 1.0)
    r = P.mpool.tile([128, 9, T0, 8], BF16, name=f"r{tag}", tag=f"r{tag}",
                     bufs=1)
    with nc.allow_low_precision(reason="sigmoid output consumed in bf16"):
        nc.vector.reciprocal(r[:, 0:ss], e[:, 0:ss])
    return r


def _consume_phases(nc, P, tch, U_t, rawt, xy, OUT_d):
    """Returns a list of phase closures for chunk tch's routing."""
    s0, ss = S0S[tch], CS[tch]
    Uf = U_t[:].rearrange("p s i pa m -> p s i (pa m)")      # [128, 9, 4, 256]

    def coord_add():
        # U[..., pa=0, (cp, a, b=k, tp)] += xy_k * raw  (raw m = (cp,a,tp))
        for k in range(2):
            tmpc = P.mpool.tile([128, 9, T0, 32], BF16, name=f"tmpc{k}",
                                tag="tmpc")
            xyb = xy[:, s0:s0 + ss, k].unsqueeze(2).unsqueeze(3) \
                .broadcast_to([128, ss, T0, 32])
            nc.vector.tensor_tensor(tmpc[:, 0:ss], rawt[:, 0:ss], xyb,
                                    op=AL.mult)
            usl = U_t[:, 0:ss, :, 0].rearrange(
                "p s i (cp a b tp) -> p (s i) cp a b tp", a=4, b=4, tp=2
            )[:, :, :, :, k, :]
            tmpv = tmpc[:, 0:ss].rearrange(
                "p s i (cp a tp) -> p (s i) cp a tp", a=4, tp=2)
            nc.vector.tensor_tensor(usl, usl, tmpv, op=AL.add)

    p = P.rpool.tile([128, 9, 2, 128], BF16, name="p", tag="p", bufs=2)
    pf = p[:].rearrange("p s pa c -> p s (pa c)")
    ts1 = P.rpool.tile([128, 9, 256], BF16, name="ts1", tag="ts1", bufs=1)
    ts2 = P.rpool.tile([128, 9, 256], BF16, name="ts2", tag="ts2", bufs=1)

    def sum_over_i(src):
        """src [128, 9, 4, 256] -> p."""
        nc.vector.tensor_tensor(ts1[:, 0:ss], src[:, 0:ss, 0],
                                src[:, 0:ss, 1], op=AL.add)
        nc.vector.tensor_tensor(ts2[:, 0:ss], src[:, 0:ss, 2],
                                src[:, 0:ss, 3], op=AL.add)
        nc.vector.tensor_tensor(pf[:, 0:ss], ts1[:, 0:ss], ts2[:, 0:ss],
                                op=AL.add)

    w = P.rpool.tile([128, 9, T0, 256], BF16, name="w", tag="w", bufs=2)

    def mult_w_by_p():
        pb = pf[:, 0:ss].unsqueeze(2).broadcast_to([128, ss, T0, 256])
        nc.vector.tensor_tensor(w[:, 0:ss], Uf[:, 0:ss], pb, op=AL.mult)

    def mult_w_by_r(r):
        # r [128, 9, 4i, 8t]; broadcast over (a, b) within each (cp, tp)
        rb = r[:, 0:ss].rearrange("p s i (cp tp) -> p (s i) cp tp", tp=2)
        rb = rb.unsqueeze(3).broadcast_to([128, ss * T0, 4, 16, 2])
        uv = Uf[:, 0:ss].rearrange(
            "p s i (pa cp ab tp) -> p (s i) pa cp ab tp", pa=2, cp=4, tp=2)
        wv = w[:, 0:ss].rearrange(
            "p s i (pa cp ab tp) -> p (s i) pa cp ab tp", pa=2, cp=4, tp=2)
        for pa in range(2):
            nc.vector.tensor_tensor(wv[:, :, pa], uv[:, :, pa], rb,
                                    op=AL.mult)

    def dots(tag):
        """z-reduce w -> ar view [128, ss, 4i, 2pa, 8t]; all on Vector."""
        Ga = ss * T0 * 2 * 4
        src = w[:, 0:ss].rearrange(
            "p s i (pa cp a b tp) -> p (s i pa cp) a b tp",
            pa=2, cp=4, a=4, tp=2)
        m4 = _tree4(nc, nc.vector, P, src, Ga, AL.add, f"d{tag}", 'b')
        return m4[:, 0:Ga].rearrange(
            "p (s i pa cp) tp -> p s i pa (cp tp)", i=T0, pa=2, cp=4)

    bacc_t = P.rpool.tile([128, 9, T0, 8], BF16, name="bacc", tag="bacc",
                          bufs=2)
    sfp1 = P.mpool.tile([128, 9, 8], BF16, name="sfp1", tag="sfp1", bufs=1)
    sfa1 = P.mpool.tile([128, 9, 8], BF16, name="sfa1", tag="sfa1", bufs=1)
    sfp2 = P.mpool.tile([128, 9, 8], BF16, name="sfp2", tag="sfp2", bufs=1)
    sfa2 = P.mpool.tile([128, 9, 8], BF16, name="sfa2", tag="sfa2", bufs=1)
    sfp3 = P.mpool.tile([128, 9, 8], BF16, name="sfp3", tag="sfp3", bufs=1)
    sfa3 = P.mpool.tile([128, 9, 8], BF16, name="sfa3", tag="sfa3", bufs=1)

    def routstep(arv, sfp, sfa, first, tag):
        sfpb = sfp[:, 0:ss].unsqueeze(2).broadcast_to([128, ss, T0, 8])
        sfab = sfa[:, 0:ss].unsqueeze(2).broadcast_to([128, ss, T0, 8])
        ta = P.mpool.tile([128, 9, T0, 8], BF16, name=f"ta{tag}", tag="rta")
        tb = P.mpool.tile([128, 9, T0, 8], BF16, name=f"tb{tag}", tag="rtb")
        nc.vector.tensor_tensor(ta[:, 0:ss], arv[:, :, :, 0], sfpb,
                                op=AL.mult)
        nc.vector.tensor_tensor(tb[:, 0:ss], arv[:, :, :, 1], sfab,
                                op=AL.mult)
        if first:
            nc.vector.tensor_tensor(bacc_t[:, 0:ss], ta[:, 0:ss],
                                    tb[:, 0:ss], op=AL.mult)
        else:
            nc.vector.tensor_tensor(ta[:, 0:ss], ta[:, 0:ss], tb[:, 0:ss],
                                    op=AL.mult)
            nc.vector.tensor_tensor(bacc_t[:, 0:ss], bacc_t[:, 0:ss],
                                    ta[:, 0:ss], op=AL.add)

    st = {}

    def ph0():
        # iter 1 head (r = 0.5 folded into app scalings; psquash scale-free)
        coord_add()
        sum_over_i(Uf)
        _stats(nc, P, p, ss, True, f"1_{tch}", sfp1, sfa1)

    def ph1():
        mult_w_by_p()
        routstep(dots("1"), sfp1, sfa1, True, "1")
        st["r2"] = _sigmoid(nc, P, bacc_t, ss, f"2_{tch}")

    def ph2():
        mult_w_by_r(st["r2"])
        sum_over_i(w[:])
        _stats(nc, P, p, ss, False, f"2_{tch}", sfp2, sfa2)

    def ph3():
        mult_w_by_p()
        routstep(dots("2"), sfp2, sfa2, False, "2")
        st["cR"] = _sigmoid(nc, P, bacc_t, ss, f"3_{tch}")

    def ph4():
        mult_w_by_r(st["cR"])
        sum_over_i(w[:])
        _stats(nc, P, p, ss, False, f"3_{tch}", sfp3, sfa3)
        v3 = P.vpool.tile([128, 9, 2, 128], BF16, name=f"v3_{tch}", tag="v3")
        for pa, sf in ((0, sfp3), (1, sfa3)):
            pv = p[:, 0:ss, pa].rearrange("p s (cp ab tp) -> p s cp ab tp",
                                          cp=4, tp=2)
            sfb = sf[:, 0:ss].rearrange("p s (cp tp) -> p s cp tp", tp=2)
            sfb = sfb.unsqueeze(3).broadcast_to([128, ss, 4, 16, 2])
            ov = v3[:, 0:ss, pa].rearrange("p s (cp ab tp) -> p s cp ab tp",
                                           cp=4, tp=2)
            nc.vector.tensor_tensor(ov, pv, sfb, op=AL.mult)
        nc.sync.dma_start(OUT_d[:, s0:s0 + ss], v3[:, 0:ss])

    return [ph0, ph1, ph2, ph3, ph4]


def _build_nc():
    nc = bacc.Bacc(None)
    P_d = nc.dram_tensor("patches", [96, T0, NBLK, 4, Z], BF16,
                         kind="ExternalInput")
    CW_d = nc.dram_tensor("convw", [96, T0, 32, 8], BF16, kind="ExternalInput")
    TW_d = nc.dram_tensor("tw", [128, T0, 2, 128], BF16, kind="ExternalInput")
    RW_d = nc.dram_tensor("rw", [128, 4, 32], BF16, kind="ExternalInput")
    KA_d = nc.dram_tensor("ka", [128, T0], F32, kind="ExternalInput")
    XY_d = nc.dram_tensor("xy", [128, NBLK, 2], F32, kind="ExternalInput")
    OUT_d = nc.dram_tensor("out", [128, NBLK, 2, 128], BF16,
                           kind="ExternalOutput")

    with tile.TileContext(nc) as tc:
        with (
            tc.tile_pool(name="const", bufs=1) as cpool,
            tc.tile_pool(name="pload", bufs=2) as ppool,
            tc.tile_pool(name="xbuf", bufs=2) as xpool,
            tc.tile_pool(name="stage", bufs=2) as spool,
            tc.tile_pool(name="ubig", bufs=2) as upool,
            tc.tile_pool(name="rscr", bufs=1) as rpool,
            tc.tile_pool(name="small", bufs=2) as mpool,
            tc.tile_pool(name="vout", bufs=2) as vpool,
            tc.tile_pool(name="ps_cv", bufs=2, space="PSUM") as pscv,
            tc.tile_pool(name="ps_uh", bufs=4, space="PSUM") as psuh,
            tc.tile_pool(name="ps_rw", bufs=2, space="PSUM") as psrw,
        ):
            P = _Pools()
            P.ppool, P.xpool, P.spool, P.upool = ppool, xpool, spool, upool
            P.rpool, P.mpool, P.vpool = rpool, mpool, vpool
            P.pscv, P.psuh, P.psrw = pscv, psuh, psrw

            cw = cpool.tile([96, T0, 32, 8], BF16, name="cw")
            nc.sync.dma_start(cw[:], CW_d[:])
            tw = cpool.tile([128, T0, 2, 128], BF16, name="tw")
            nc.sync.dma_start(tw[:], TW_d[:])
            rw = cpool.tile([128, 4, 32], BF16, name="rw")
            nc.sync.dma_start(rw[:], RW_d[:])
            kab = cpool.tile([128, T0], F32, name="kab")
            nc.sync.dma_start(kab[:], KA_d[:])
            xy = cpool.tile([128, NBLK, 2], F32, name="xy")
            nc.sync.dma_start(xy[:], XY_d[:])
            P.epsb = cpool.tile([128, 1], F32, name="epsb")
            nc.vector.memset(P.epsb[:], 1e-9)
            P.ln8b = cpool.tile([128, 1], F32, name="ln8b")
            nc.vector.memset(P.ln8b[:], LN_EIGHTH)
            P.zerob = cpool.tile([128, 1], F32, name="zerob")
            nc.vector.memset(P.zerob[:], 0.0)
            # Pre-load the combined Ln+Exp activation table so the
            # insert_act_table_loads pass sees it on every path (the greedy
            # chooser would otherwise thrash natural_log <-> exp_and_others).
            _preload = mybir.InstLoadActFuncSet(
                name=nc.get_next_instruction_name(), ins=[], outs=[],
                act_func_set_id=6)
            nc.scalar.add_instruction(_preload)

            # software pipeline: interleave produce(c+1) i-blocks with
            # consume(c) phases so no engine queue gets head-of-line blocked
            # behind a full chunk of foreign work.
            NCH = len(CS)
            chunks = {0: _alloc_chunk(P, 0)}
            nxt = _dma_pt(nc, P, 0, 0, P_d)
            for i in range(T0):
                pt, nxt = nxt, (_dma_pt(nc, P, 0, i + 1, P_d)
                                if i + 1 < T0 else _dma_pt(nc, P, 1, 0, P_d))
                _produce_i(nc, P, 0, i, P_d, cw, tw, rw, kab, *chunks[0], pt)
            phl = {0: _consume_phases(nc, P, 0, *chunks[0], xy, OUT_d)}
            phl[0][0]()
            # phase-shifted pipeline: ph0(c+1) is issued before ph4(c) so
            # the next chunk's independent Vector work fills the sigmoid /
            # stats cross-engine stalls of the current chunk's tail.
            for c in range(NCH):
                if c + 1 < NCH:
                    chunks[c + 1] = _alloc_chunk(P, c + 1)
                    phl[c + 1] = _consume_phases(nc, P, c + 1,
                                                 *chunks[c + 1], xy, OUT_d)
                    for i in range(T0):
                        pt, nxt = nxt, (
                            _dma_pt(nc, P, c + 1, i + 1, P_d)
                            if i + 1 < T0 else
                            (_dma_pt(nc, P, c + 2, 0, P_d)
                             if c + 2 < NCH else None))
                        _produce_i(nc, P, c + 1, i, P_d, cw, tw, rw, kab,
                                   *chunks[c + 1], pt)
                        if i < 3:
                            phl[c][i + 1]()
                    phl[c + 1][0]()
                    phl[c][4]()
                else:
                    for k in (1, 2, 3, 4):
                        phl[c][k]()
    nc.finalize()
    return nc


_NC_CACHE = None


def _get_nc():
    global _NC_CACHE
    if _NC_CACHE is None:
        _NC_CACHE = _build_nc()
    return _NC_CACHE


def kernel(input_tensor, W_conv, W_pos, W_app, b_app):
    input_tensor = np.asarray(input_tensor, np.float32)
    CW, TW2, RW3, KAB = _build_weights(np.asarray(W_conv, np.float32),
                                       np.asarray(W_pos, np.float32),
                                       np.asarray(W_app, np.float32),
                                       np.asarray(b_app, np.float32))
    N = input_tensor.shape[0]
    full_pad = np.pad(input_tensor, ((0, 0), (0, 0), (0, 0), (2, 2), (2, 2)))
    bf = np.float16
    in_maps = []
    for c in range(8):
        n, hh = c // 2, c % 2
        sl = full_pad[n, :, :, 48 * hh:48 * hh + 52, :]
        in_maps.append({
            "patches": _build_patches(sl).astype(bf),
            "convw": CW.astype(bf),
            "tw": TW2.astype(bf),
            "rw": RW3.astype(bf),
            "ka": KAB.astype(np.float32),
            "xy": _pixel_coords(hh).astype(np.float32),
        })
    nc = _get_nc()
    kres = run_bass_kernel_spmd(nc, in_maps, core_ids=list(range(8)))
    global LAST_RESULT
    LAST_RESULT = kres
    res = kres.results
    # unscramble: out dram [128px=(j,pi,pj), blk36, pa2, m128=(cp,a,b,tp)]
    blk = np.arange(NBLK)
    j = np.arange(4)
    pi = np.arange(4)
    pj = np.arange(8)
    hmap = (4 * (blk // 3))[:, None, None, None] + pi[None, None, :, None]
    hmap = np.broadcast_to(hmap, (NBLK, 4, 4, 8)).ravel()
    wmap = (32 * (blk % 3))[:, None, None, None] + 8 * j[None, :, None, None] \
        + pj[None, None, None, :]
    wmap = np.broadcast_to(wmap, (NBLK, 4, 4, 8)).ravel()
    out = np.zeros((N, T1, Z, H, W), np.float32)
    for c in range(8):
        n, hh = c // 2, c % 2
        v = np.asarray(res[c]["out"]).astype(np.float32)
        v = v.reshape(128, NBLK, 2, 4, 4, 4, 2)
        # m=(cp,a,b,tp) -> [pa, cp, tp, a, b, blk, px] -> [pa, t, z, blk*px]
        vv = v.transpose(2, 3, 6, 4, 5, 1, 0).reshape(2, 8, 16, NBLK * 128)
        img = np.zeros((2, 8, 16, HC, W), np.float32)
        img[:, :, :, hmap, wmap] = vv
        for pa in range(2):
            out[n, :, pa * 16:pa * 16 + 16, 48 * hh:48 * hh + 48] = img[pa]
    return out


# revision 58
# speedup vs baseline: 1.0997x; 1.0997x over previous
"""Trainium2 Bass kernel for nn_Caps2dMatwo (capsule conv + matwo dual routing).

Final (v25): v2 m-layout (cp, a, b, tp; cp forced outer by the conv-channel
coupling), chunk schedule [5,6,9,9,7], phase-shifted software pipeline
(ph0 of chunk c+1 issued before ph4 of chunk c, double-buffered scratch),
patch-DMA prefetch one block ahead, and the ENTIRE routing chain (coord,
sums, trees, routstep, stats side-chain, sigmoid) on the Vector engine --
on this FIFO-queued machine, any cross-engine hop in the serial chain
stalls everything behind it, so consolidation beat load-balancing in four
separate measured steps. ACT runs only table-bound activations (Square/
Ln/Exp) and PSUM drains; GpSimd runs nothing in consume. bf16 sigmoid/
bacc intermediates; reciprocal_approx_fast for the squash denominator.

Sharding: 8 cores = (batch n: 4) x (h-half: 2); each core computes a 48-row
slab of one batch element independently (halo via host padding).

Layouts (per core):
  patches P [96, T0, 36blk, 4j, 32z]  (host im2col; blocks of 128 px)
  conv psum X [(j,z)=128, (s2, pix32, co8)]
  transform m-index = 32*cp + 8*a + 2*b + tp  (t = 2cp+tp, z_out = 4a+b)
  raw m = 8*cp + 2*a + tp
  U (pixel-major) [128px, 9s, 4i, 2pa, 128m]; raw [128px, 9s, 4i, 32]
  routing all on-chip; v3 [128px, 9s, 2pa, 128m] -> DRAM bf16, host unscramble.
"""
import sys
import numpy as np

sys.path.insert(0, "/opt/trn_rl_repo")

import concourse.bass as bass
import concourse.bacc as bacc
import concourse.mybir as mybir
from concourse import tile
from concourse.bass_utils import run_bass_kernel_spmd
import ml_dtypes

BF16 = mybir.dt.float16
F32 = mybir.dt.float32
AL = mybir.AluOpType
AF = mybir.ActivationFunctionType

T0, T1, Z, H, W, HC = 4, 8, 32, 96, 96, 48
NBLK = 36
LN_EIGHTH = float(np.log(0.125))
CS = [5, 6, 9, 9, 7]                  # chunk sizes (s-blocks of 128 px)
S0S = [0, 5, 11, 20, 29]              # chunk offsets


# ----------------------------------------------------------------------------
# host-side weight/layout construction
# ----------------------------------------------------------------------------

def _build_weights(W_conv, W_pos, W_app, b_app):
    CW = np.zeros((96, T0, 32, 8), np.float32)
    for hi in range(8):
        for wi in range(12):
            for pi in range(4):
                for pj in range(8):
                    dy, dx = hi - pi, wi - pj
                    if 0 <= dy < 5 and 0 <= dx < 5:
                        CW[hi * 12 + wi, :, pi * 8 + pj, :] = W_conv[:, dy, dx, 0, :]

    m_pos = np.stack([W_pos[i].reshape(T1, 4, 4) for i in range(T0)])
    m_app = np.stack([W_app[i].reshape(T1, 4, 4) for i in range(T0)])
    nrm = np.sqrt(np.maximum((m_pos ** 2).sum(axis=2, keepdims=True), 1e-12))
    m_pos = m_pos / nrm

    # TW2[32j+z, i, pa, 32cp+8a+2b+tp] = M[i, 2cp+tp, c, b],  z = 16tp+4a+c
    TW2 = np.zeros((128, T0, 2, 128), np.float32)
    for i in range(T0):
        for pa, M in ((0, m_pos), (1, m_app)):
            blk = np.zeros((32, 128), np.float32)
            for cp in range(4):
                for tp in range(2):
                    t = 2 * cp + tp
                    for a in range(4):
                        for b in range(4):
                            m = 32 * cp + 8 * a + 2 * b + tp
                            for c in range(4):
                                blk[16 * tp + 4 * a + c, m] = M[i, t, c, b]
            for j in range(4):
                TW2[32 * j:32 * j + 32, i, pa] = blk

    # RW3[32j + 16tp+4a+3, cp, 8cp+2a+tp] = 1
    RW3 = np.zeros((128, 4, 32), np.float32)
    for cp in range(4):
        for tp in range(2):
            for a in range(4):
                for j in range(4):
                    RW3[32 * j + 16 * tp + 4 * a + 3, cp, 8 * cp + 2 * a + tp] = 1.0

    # KAB[32cp+8a+2b+tp, i] = b_app[i, 2cp+tp] * sum_c m_app[i, 2cp+tp, c, b]
    KAB = np.zeros((128, T0), np.float32)
    for i in range(T0):
        for cp in range(4):
            for tp in range(2):
                t = 2 * cp + tp
                for a in range(4):
                    for b in range(4):
                        m = 32 * cp + 8 * a + 2 * b + tp
                        KAB[m, i] = b_app[i, t] * m_app[i, t, :, b].sum()
    return CW, TW2, RW3, KAB


_PH = np.arange(NBLK) // 3
_B3 = np.arange(NBLK) % 3
_HIDX = (4 * _PH)[:, None] + np.arange(8)[None, :]
_PWJ = (4 * _B3)[:, None] + np.arange(4)[None, :]
_WIDX = (8 * _PWJ)[:, :, None] + np.arange(12)[None, None, :]


def _build_patches(pad):
    g = pad[:, :, _HIDX[:, None, :, None], _WIDX[:, :, None, :]]
    return np.ascontiguousarray(
        g.transpose(4, 5, 0, 2, 3, 1).reshape(96, T0, NBLK, 4, Z))


def _pixel_coords(hh):
    xs = np.zeros((128, NBLK, 2), np.float32)
    for b in range(NBLK):
        ph, b3 = b // 3, b % 3
        for j in range(4):
            for pi in range(4):
                for pj in range(8):
                    part = j * 32 + pi * 8 + pj
                    xs[part, b, 0] = (8 * (4 * b3 + j) + pj) / W
                    xs[part, b, 1] = (4 * ph + pi + 48 * hh) / H
    return xs


# ----------------------------------------------------------------------------
# device kernel
# ----------------------------------------------------------------------------

class _Pools:
    pass


def _alloc_chunk(P, tch):
    U_t = P.upool.tile([128, 9, T0, 2, 128], BF16, name=f"U{tch}", tag="ubig",
                       bufs=2)
    rawt = P.spool.tile([128, 9, T0, 32], BF16, name=f"rawt{tch}", tag="rawt",
                        bufs=2)
    return U_t, rawt


def _dma_pt(nc, P, tch, i, P_d):
    """Patch load for block (tch, i) — issued one block early so the conv
    never waits on HBM."""
    s0, ss = S0S[tch], CS[tch]
    pt = P.ppool.tile([96, 9, 4, Z], BF16, name="pt", tag="pt")
    nc.sync.dma_start(pt[:, 0:ss], P_d[:, i, s0:s0 + ss])
    return pt


def _produce_i(nc, P, tch, i, P_d, cw, tw, rw, kab, U_t, rawt, pt):
    """conv + transform + raw for one input-capsule block of chunk tch."""
    s0, ss = S0S[tch], CS[tch]
    xsb = P.xpool.tile([128, 9, 32, 8], BF16, name="xsb", tag="xsb")
    # conv: ss matmuls, drained in pairs (ACT)
    g = 0
    while g < ss:
        ns = min(2, ss - g)
        cps = P.pscv.tile([128, 2, 256], F32, name="cps", tag="cv")
        for s2 in range(ns):
            nc.tensor.matmul(
                cps[:, s2],
                pt[:, g + s2].rearrange("p j z -> p (j z)"),
                cw[:, i].rearrange("p f c -> p (f c)"),
                start=True, stop=True)
        nc.scalar.copy(
            xsb[:, g:g + ns].rearrange("p s f c -> p (s f c)"),
            cps[:, 0:ns].rearrange("p s f -> p (s f)"))
        g += ns
    # transform pos/app: per (pa, rp) psum tile [128, (ss, pix32)]
    for pa in range(2):
        stg = P.spool.tile([128, 9, 4, 32], BF16, name=f"stg{pa}",
                           tag=f"stg{pa}")
        for rp in range(4):
            ups = P.psuh.tile([128, 9, 32], F32, name="ups", tag="uh")
            for cp in range(4):
                nc.tensor.matmul(
                    ups[32 * cp:32 * cp + 32, 0:ss],
                    tw[32 * rp:32 * rp + 32, i, pa, 32 * cp:32 * cp + 32],
                    xsb[32 * rp:32 * rp + 32, 0:ss, :, pa * 4 + cp],
                    start=True, stop=True, tile_position=(32 * rp, 32 * cp))
            if pa == 0:
                nc.scalar.copy(stg[:, 0:ss, rp, :], ups[:, 0:ss])
            else:
                # drain + bias add (KAB is per-partition in m-space)
                nc.scalar.activation(stg[:, 0:ss, rp, :], ups[:, 0:ss],
                                     AF.Identity, bias=kab[:, i:i + 1])
        nc.sync.dma_start(
            U_t[:, 0:ss, i, pa, :],
            stg[:, 0:ss].rearrange("p s j f -> p (s j f)"), transpose=True)
    # raw extraction (pos conv channels, c=3 picks), accumulated over cp
    rstg = P.spool.tile([32, 9, 4, 32], BF16, name="rstg", tag="rstg")
    for rp in range(4):
        rps = P.psrw.tile([32, 9, 32], F32, name="rps", tag="rw")
        for cp in range(4):
            nc.tensor.matmul(
                rps[:, 0:ss].rearrange("p s x -> p (s x)"),
                rw[32 * rp:32 * rp + 32, cp],
                xsb[32 * rp:32 * rp + 32, 0:ss, :, cp],
                start=(cp == 0), stop=(cp == 3),
                tile_position=(32 * rp, 0))
        nc.scalar.copy(rstg[:, 0:ss, rp, :], rps[:, 0:ss])
    nc.sync.dma_start(
        rawt[:, 0:ss, i, :],
        rstg[:, 0:ss].rearrange("p s j f -> p (s j f)"), transpose=True)


def _tree4(nc, eng, P, src4d, G, op, pref, sz, out_dt=BF16):
    """src4d: AP [128, G, 4a, 4b, 2tp] -> [128, G, 2] tile (op-reduce over
    the middle 4x4) on engine `eng`. sz: tile-size class ('b' big/'s' small)."""
    m1 = P.mpool.tile([128, 288 if sz == 'b' else 36, 2, 4, 2], BF16,
                      name=f"{pref}m1", tag=f"{sz}m1")
    eng.tensor_tensor(m1[:, 0:G], src4d[:, :, 0:2], src4d[:, :, 2:4], op=op)
    m2 = P.mpool.tile([128, 288 if sz == 'b' else 36, 4, 2], BF16,
                      name=f"{pref}m2", tag=f"{sz}m2")
    eng.tensor_tensor(m2[:, 0:G], m1[:, 0:G, 0], m1[:, 0:G, 1], op=op)
    m3 = P.mpool.tile([128, 288 if sz == 'b' else 36, 2, 2], BF16,
                      name=f"{pref}m3", tag=f"{sz}m3")
    eng.tensor_tensor(m3[:, 0:G], m2[:, 0:G, 0:2], m2[:, 0:G, 2:4], op=op)
    m4 = P.mpool.tile([128, 288 if sz == 'b' else 36, 2], out_dt,
                      name=f"{pref}m4", tag=f"{sz}m4")
    eng.tensor_tensor(m4[:, 0:G], m3[:, 0:G, 0], m3[:, 0:G, 1], op=op)
    return m4


def _stats(nc, P, p, ss, iter1, tag, sfp, sfa):
    """p: tile [128, 9, 2, 128] (m = (cp,a,b,tp)). Writes per-(s,t) scale
    factors sfp/sfa [128, 9, 8] bf16 (valid [:, :ss]); pos tree on Vector,
    app tree on GpSimd."""
    G = ss * 4
    # pos: sfp = 1/max|p_pos| = exp(-0.5 ln(max p^2))  (scale-invariant)
    pab = P.mpool.tile([128, 9, 128], BF16, name=f"pab{tag}", tag="pab")
    nc.scalar.activation(pab[:, 0:ss], p[:, 0:ss, 0], AF.Square)
    pv = pab[:, 0:ss].rearrange("p s (cp a b tp) -> p (s cp) a b tp",
                                a=4, b=4, tp=2)
    mx = _tree4(nc, nc.vector, P, pv, G, AL.max, f"mx{tag}", 's', out_dt=F32)
    lnp = P.mpool.tile([128, 36, 2], F32, name=f"lnp{tag}", tag="lnp")
    nc.scalar.activation(lnp[:, 0:G].rearrange("p g tp -> p (g tp)"),
                         mx[:, 0:G].rearrange("p g tp -> p (g tp)"),
                         AF.Ln, bias=P.zerob[:, 0:1])
    nc.scalar.activation(
        sfp[:, 0:ss].rearrange("p s t -> p (s t)"),
        lnp[:, 0:G].rearrange("p g tp -> p (g tp)"),
        AF.Exp, scale=-0.5, bias=P.zerob[:, 0:1])
    # app: n2 = sum p_app^2 (x0.25 on iter1 via activation scale);
    # sfa = n2'/(1+n2') / sqrt(n2'+eps)  (x0.5 iter1 via exp bias)
    asq = P.mpool.tile([128, 9, 128], BF16, name=f"asq{tag}", tag="asq")
    nc.scalar.activation(asq[:, 0:ss], p[:, 0:ss, 1], AF.Square)
    av = asq[:, 0:ss].rearrange("p s (cp a b tp) -> p (s cp) a b tp",
                                a=4, b=4, tp=2)
    n2 = _tree4(nc, nc.vector, P, av, G, AL.add, f"n2{tag}", 's', out_dt=F32)
    scl = 0.25 if iter1 else 1.0
    g = P.mpool.tile([128, 36, 2], F32, name=f"g{tag}", tag="lga")
    nc.scalar.activation(g[:, 0:G].rearrange("p g tp -> p (g tp)"),
                         n2[:, 0:G].rearrange("p g tp -> p (g tp)"),
                         AF.Ln, bias=P.epsb[:, 0:1], scale=scl)
    h = P.mpool.tile([128, 36, 2], BF16, name=f"h{tag}", tag="h")
    nc.scalar.activation(h[:, 0:G].rearrange("p g tp -> p (g tp)"),
                         g[:, 0:G].rearrange("p g tp -> p (g tp)"),
                         AF.Exp, scale=-0.5,
                         bias=P.ln8b[:, 0:1] if iter1 else P.zerob[:, 0:1])
    den = P.mpool.tile([128, 36, 2], F32, name=f"den{tag}", tag="den")
    nc.vector.tensor_scalar(den[:, 0:G], n2[:, 0:G], scl, 1.0,
                            op0=AL.mult, op1=AL.add)
    rec = P.mpool.tile([128, 36, 2], F32, name=f"rec{tag}", tag="rec")
    nc.vector.reciprocal_approx_fast(rec[:, 0:G], den[:, 0:G])
    u1 = P.mpool.tile([128, 36, 2], F32, name=f"u1{tag}", tag="u1")
    nc.vector.tensor_tensor(u1[:, 0:G], n2[:, 0:G], rec[:, 0:G], op=AL.mult)
    nc.vector.tensor_tensor(
        sfa[:, 0:ss],
        u1[:, 0:G].rearrange("p (s cp) tp -> p s (cp tp)", cp=4),
        h[:, 0:G].rearrange("p (s cp) tp -> p s (cp tp)", cp=4), op=AL.mult)


def _sigmoid(nc, P, bacc_t, ss, tag):
    """r = 1/(1+exp(-b)) -> bf16 [128, 9, T0, 8] (valid [:, :ss])."""
    e = P.mpool.tile([128, 9, T0, 8], BF16, name=f"e{tag}", tag="sge")
    nc.scalar.activation(e[:, 0:ss].rearrange("p s i t -> p (s i t)"),
                         bacc_t[:, 0:ss].rearrange("p s i t -> p (s i t)"),
                         AF.Exp, scale=-1.0, bias=P.zerob[:, 0:1])
    nc.vector.tensor_scalar_add(e[:, 0:ss], e[:, 0:ss], 1.0)
    r = P.mpool.tile([128, 9, T0, 8], BF16, name=f"r{tag}", tag=f"r{tag}",
                     bufs=1)
    with nc.allow_low_precision(reason="sigmoid output consumed in bf16"):
        nc.vector.reciprocal(r[:, 0:ss], e[:, 0:ss])
    return r


def _consume_phases(nc, P, tch, U_t, rawt, xy, OUT_d):
    """Returns a list of phase closures for chunk tch's routing."""
    s0, ss = S0S[tch], CS[tch]
    Uf = U_t[:].rearrange("p s i pa m -> p s i (pa m)")      # [128, 9, 4, 256]

    def coord_add():
        # U[..., pa=0, (cp, a, b=k, tp)] += xy_k * raw  (raw m = (cp,a,tp))
        for k in range(2):
            tmpc = P.mpool.tile([128, 9, T0, 32], BF16, name=f"tmpc{k}",
                                tag="tmpc")
            xyb = xy[:, s0:s0 + ss, k].unsqueeze(2).unsqueeze(3) \
                .broadcast_to([128, ss, T0, 32])
            nc.vector.tensor_tensor(tmpc[:, 0:ss], rawt[:, 0:ss], xyb,
                                    op=AL.mult)
            usl = U_t[:, 0:ss, :, 0].rearrange(
                "p s i (cp a b tp) -> p (s i) cp a b tp", a=4, b=4, tp=2
            )[:, :, :, :, k, :]
            tmpv = tmpc[:, 0:ss].rearrange(
                "p s i (cp a tp) -> p (s i) cp a tp", a=4, tp=2)
            nc.vector.tensor_tensor(usl, usl, tmpv, op=AL.add)

    p = P.rpool.tile([128, 9, 2, 128], BF16, name="p", tag="p", bufs=2)
    pf = p[:].rearrange("p s pa c -> p s (pa c)")
    ts1 = P.rpool.tile([128, 9, 256], BF16, name="ts1", tag="ts1", bufs=1)
    ts2 = P.rpool.tile([128, 9, 256], BF16, name="ts2", tag="ts2", bufs=1)

    def sum_over_i(src):
        """src [128, 9, 4, 256] -> p."""
        nc.vector.tensor_tensor(ts1[:, 0:ss], src[:, 0:ss, 0],
                                src[:, 0:ss, 1], op=AL.add)
        nc.vector.tensor_tensor(ts2[:, 0:ss], src[:, 0:ss, 2],
                                src[:, 0:ss, 3], op=AL.add)
        nc.vector.tensor_tensor(pf[:, 0:ss], ts1[:, 0:ss], ts2[:, 0:ss],
                                op=AL.add)

    w = P.rpool.tile([128, 9, T0, 256], BF16, name="w", tag="w", bufs=2)

    def mult_w_by_p():
        pb = pf[:, 0:ss].unsqueeze(2).broadcast_to([128, ss, T0, 256])
        nc.vector.tensor_tensor(w[:, 0:ss], Uf[:, 0:ss], pb, op=AL.mult)

    def mult_w_by_r(r):
        # r [128, 9, 4i, 8t]; broadcast over (a, b) within each (cp, tp)
        rb = r[:, 0:ss].rearrange("p s i (cp tp) -> p (s i) cp tp", tp=2)
        rb = rb.unsqueeze(3).broadcast_to([128, ss * T0, 4, 16, 2])
        uv = Uf[:, 0:ss].rearrange(
            "p s i (pa cp ab tp) -> p (s i) pa cp ab tp", pa=2, cp=4, tp=2)
        wv = w[:, 0:ss].rearrange(
            "p s i (pa cp ab tp) -> p (s i) pa cp ab tp", pa=2, cp=4, tp=2)
        for pa in range(2):
            nc.vector.tensor_tensor(wv[:, :, pa], uv[:, :, pa], rb,
                                    op=AL.mult)

    def dots(tag):
        """z-reduce w -> ar view [128, ss, 4i, 2pa, 8t]; all on Vector."""
        Ga = ss * T0 * 2 * 4
        src = w[:, 0:ss].rearrange(
            "p s i (pa cp a b tp) -> p (s i pa cp) a b tp",
            pa=2, cp=4, a=4, tp=2)
        m4 = _tree4(nc, nc.vector, P, src, Ga, AL.add, f"d{tag}", 'b')
        return m4[:, 0:Ga].rearrange(
            "p (s i pa cp) tp -> p s i pa (cp tp)", i=T0, pa=2, cp=4)

    bacc_t = P.rpool.tile([128, 9, T0, 8], BF16, name="bacc", tag="bacc",
                          bufs=2)
    sfp1 = P.mpool.tile([128, 9, 8], BF16, name="sfp1", tag="sfp1", bufs=1)
    sfa1 = P.mpool.tile([128, 9, 8], BF16, name="sfa1", tag="sfa1", bufs=1)
    sfp2 = P.mpool.tile([128, 9, 8], BF16, name="sfp2", tag="sfp2", bufs=1)
    sfa2 = P.mpool.tile([128, 9, 8], BF16, name="sfa2", tag="sfa2", bufs=1)
    sfp3 = P.mpool.tile([128, 9, 8], BF16, name="sfp3", tag="sfp3", bufs=1)
    sfa3 = P.mpool.tile([128, 9, 8], BF16, name="sfa3", tag="sfa3", bufs=1)

    def routstep(arv, sfp, sfa, first, tag):
        sfpb = sfp[:, 0:ss].unsqueeze(2).broadcast_to([128, ss, T0, 8])
        sfab = sfa[:, 0:ss].unsqueeze(2).broadcast_to([128, ss, T0, 8])
        ta = P.mpool.tile([128, 9, T0, 8], BF16, name=f"ta{tag}", tag="rta")
        tb = P.mpool.tile([128, 9, T0, 8], BF16, name=f"tb{tag}", tag="rtb")
        nc.vector.tensor_tensor(ta[:, 0:ss], arv[:, :, :, 0], sfpb,
                                op=AL.mult)
        nc.vector.tensor_tensor(tb[:, 0:ss], arv[:, :, :, 1], sfab,
                                op=AL.mult)
        if first:
            nc.vector.tensor_tensor(bacc_t[:, 0:ss], ta[:, 0:ss],
                                    tb[:, 0:ss], op=AL.mult)
        else:
            nc.vector.tensor_tensor(ta[:, 0:ss], ta[:, 0:ss], tb[:, 0:ss],
                                    op=AL.mult)
            nc.vector.tensor_tensor(bacc_t[:, 0:ss], bacc_t[:, 0:ss],
                                    ta[:, 0:ss], op=AL.add)

    st = {}

    def ph0():
        # iter 1 head (r = 0.5 folded into app scalings; psquash scale-free)
        coord_add()
        sum_over_i(Uf)
        _stats(nc, P, p, ss, True, f"1_{tch}", sfp1, sfa1)

    def ph1():
        mult_w_by_p()
        routstep(dots("1"), sfp1, sfa1, True, "1")
        st["r2"] = _sigmoid(nc, P, bacc_t, ss, f"2_{tch}")

    def ph2():
        mult_w_by_r(st["r2"])
        sum_over_i(w[:])
        _stats(nc, P, p, ss, False, f"2_{tch}", sfp2, sfa2)

    def ph3():
        mult_w_by_p()
        routstep(dots("2"), sfp2, sfa2, False, "2")
        st["cR"] = _sigmoid(nc, P, bacc_t, ss, f"3_{tch}")

    def ph4():
        mult_w_by_r(st["cR"])
        sum_over_i(w[:])
        _stats(nc, P, p, ss, False, f"3_{tch}", sfp3, sfa3)
        v3 = P.vpool.tile([128, 9, 2, 128], BF16, name=f"v3_{tch}", tag="v3")
        for pa, sf in ((0, sfp3), (1, sfa3)):
            pv = p[:, 0:ss, pa].rearrange("p s (cp ab tp) -> p s cp ab tp",
                                          cp=4, tp=2)
            sfb = sf[:, 0:ss].rearrange("p s (cp tp) -> p s cp tp", tp=2)
            sfb = sfb.unsqueeze(3).broadcast_to([128, ss, 4, 16, 2])
            ov = v3[:, 0:ss, pa].rearrange("p s (cp ab tp) -> p s cp ab tp",
                                           cp=4, tp=2)
            nc.vector.tensor_tensor(ov, pv, sfb, op=AL.mult)
        nc.sync.dma_start(OUT_d[:, s0:s0 + ss], v3[:, 0:ss])

    return [ph0, ph1, ph2, ph3, ph4]


def _build_nc():
    nc = bacc.Bacc(None)
    P_d = nc.dram_tensor("patches", [96, T0, NBLK, 4, Z], BF16,
                         kind="ExternalInput")
    CW_d = nc.dram_tensor("convw", [96, T0, 32, 8], BF16, kind="ExternalInput")
    TW_d = nc.dram_tensor("tw", [128, T0, 2, 128], BF16, kind="ExternalInput")
    RW_d = nc.dram_tensor("rw", [128, 4, 32], BF16, kind="ExternalInput")
    KA_d = nc.dram_tensor("ka", [128, T0], F32, kind="ExternalInput")
    XY_d = nc.dram_tensor("xy", [128, NBLK, 2], F32, kind="ExternalInput")
    OUT_d = nc.dram_tensor("out", [128, NBLK, 2, 128], BF16,
                           kind="ExternalOutput")

    with tile.TileContext(nc) as tc:
        with (
            tc.tile_pool(name="const", bufs=1) as cpool,
            tc.tile_pool(name="pload", bufs=2) as ppool,
            tc.tile_pool(name="xbuf", bufs=2) as xpool,
            tc.tile_pool(name="stage", bufs=2) as spool,
            tc.tile_pool(name="ubig", bufs=2) as upool,
            tc.tile_pool(name="rscr", bufs=1) as rpool,
            tc.tile_pool(name="small", bufs=2) as mpool,
            tc.tile_pool(name="vout", bufs=2) as vpool,
            tc.tile_pool(name="ps_cv", bufs=2, space="PSUM") as pscv,
            tc.tile_pool(name="ps_uh", bufs=4, space="PSUM") as psuh,
            tc.tile_pool(name="ps_rw", bufs=2, space="PSUM") as psrw,
        ):
            P = _Pools()
            P.ppool, P.xpool, P.spool, P.upool = ppool, xpool, spool, upool
            P.rpool, P.mpool, P.vpool = rpool, mpool, vpool
            P.pscv, P.psuh, P.psrw = pscv, psuh, psrw

            cw = cpool.tile([96, T0, 32, 8], BF16, name="cw")
            nc.sync.dma_start(cw[:], CW_d[:])
            tw = cpool.tile([128, T0, 2, 128], BF16, name="tw")
            nc.sync.dma_start(tw[:], TW_d[:])
            rw = cpool.tile([128, 4, 32], BF16, name="rw")
            nc.sync.dma_start(rw[:], RW_d[:])
            kab = cpool.tile([128, T0], F32, name="kab")
            nc.sync.dma_start(kab[:], KA_d[:])
            xy = cpool.tile([128, NBLK, 2], F32, name="xy")
            nc.sync.dma_start(xy[:], XY_d[:])
            P.epsb = cpool.tile([128, 1], F32, name="epsb")
            nc.vector.memset(P.epsb[:], 1e-9)
            P.ln8b = cpool.tile([128, 1], F32, name="ln8b")
            nc.vector.memset(P.ln8b[:], LN_EIGHTH)
            P.zerob = cpool.tile([128, 1], F32, name="zerob")
            nc.vector.memset(P.zerob[:], 0.0)
            # Pre-load the combined Ln+Exp activation table so the
            # insert_act_table_loads pass sees it on every path (the greedy
            # chooser would otherwise thrash natural_log <-> exp_and_others).
            _preload = mybir.InstLoadActFuncSet(
                name=nc.get_next_instruction_name(), ins=[], outs=[],
                act_func_set_id=6)
            nc.scalar.add_instruction(_preload)

            # software pipeline: interleave produce(c+1) i-blocks with
            # consume(c) phases so no engine queue gets head-of-line blocked
            # behind a full chunk of foreign work.
            NCH = len(CS)
            chunks = {0: _alloc_chunk(P, 0)}
            nxt = _dma_pt(nc, P, 0, 0, P_d)
            for i in range(T0):
                pt, nxt = nxt, (_dma_pt(nc, P, 0, i + 1, P_d)
                                if i + 1 < T0 else _dma_pt(nc, P, 1, 0, P_d))
                _produce_i(nc, P, 0, i, P_d, cw, tw, rw, kab, *chunks[0], pt)
            phl = {0: _consume_phases(nc, P, 0, *chunks[0], xy, OUT_d)}
            phl[0][0]()
            # phase-shifted pipeline: ph0(c+1) is issued before ph4(c) so
            # the next chunk's independent Vector work fills the sigmoid /
            # stats cross-engine stalls of the current chunk's tail.
            for c in range(NCH):
                if c + 1 < NCH:
                    chunks[c + 1] = _alloc_chunk(P, c + 1)
                    phl[c + 1] = _consume_phases(nc, P, c + 1,
                                                 *chunks[c + 1], xy, OUT_d)
                    for i in range(T0):
                        pt, nxt = nxt, (
                            _dma_pt(nc, P, c + 1, i + 1, P_d)
                            if i + 1 < T0 else
                            (_dma_pt(nc, P, c + 2, 0, P_d)
                             if c + 2 < NCH else None))
                        _produce_i(nc, P, c + 1, i, P_d, cw, tw, rw, kab,
                                   *chunks[c + 1], pt)
                        if i < 3:
                            phl[c][i + 1]()
                    phl[c + 1][0]()
                    phl[c][4]()
                else:
                    for k in (1, 2, 3, 4):
                        phl[c][k]()
    nc.finalize()
    return nc


_NC_CACHE = None


def _get_nc():
    global _NC_CACHE
    if _NC_CACHE is None:
        _NC_CACHE = _build_nc()
    return _NC_CACHE


def kernel(input_tensor, W_conv, W_pos, W_app, b_app):
    input_tensor = np.asarray(input_tensor, np.float32)
    CW, TW2, RW3, KAB = _build_weights(np.asarray(W_conv, np.float32),
                                       np.asarray(W_pos, np.float32),
                                       np.asarray(W_app, np.float32),
                                       np.asarray(b_app, np.float32))
    N = input_tensor.shape[0]
    full_pad = np.pad(input_tensor, ((0, 0), (0, 0), (0, 0), (2, 2), (2, 2)))
    bf = np.float16
    in_maps = []
    for c in range(8):
        n, hh = c // 2, c % 2
        sl = full_pad[n, :, :, 48 * hh:48 * hh + 52, :]
        in_maps.append({
            "patches": _build_patches(sl).astype(bf),
            "convw": CW.astype(bf),
            "tw": TW2.astype(bf),
            "rw": RW3.astype(bf),
            "ka": KAB.astype(np.float32),
            "xy": _pixel_coords(hh).astype(np.float32),
        })
    nc = _get_nc()
    kres = run_bass_kernel_spmd(nc, in_maps, core_ids=list(range(8)))
    global LAST_RESULT
    LAST_RESULT = kres
    res = kres.results
    # unscramble: out dram [128px=(j,pi,pj), blk36, pa2, m128=(cp,a,b,tp)]
    blk = np.arange(NBLK)
    j = np.arange(4)
    pi = np.arange(4)
    pj = np.arange(8)
    hmap = (4 * (blk // 3))[:, None, None, None] + pi[None, None, :, None]
    hmap = np.broadcast_to(hmap, (NBLK, 4, 4, 8)).ravel()
    wmap = (32 * (blk % 3))[:, None, None, None] + 8 * j[None, :, None, None] \
        + pj[None, None, None, :]
    wmap = np.broadcast_to(wmap, (NBLK, 4, 4, 8)).ravel()
    out = np.zeros((N, T1, Z, H, W), np.float32)
    for c in range(8):
        n, hh = c // 2, c % 2
        v = np.asarray(res[c]["out"]).astype(np.float32)
        v = v.reshape(128, NBLK, 2, 4, 4, 4, 2)
        # m=(cp,a,b,tp) -> [pa, cp, tp, a, b, blk, px] -> [pa, t, z, blk*px]
        vv = v.transpose(2, 3, 6, 4, 5, 1, 0).reshape(2, 8, 16, NBLK * 128)
        img = np.zeros((2, 8, 16, HC, W), np.float32)
        img[:, :, :, hmap, wmap] = vv
        for pa in range(2):
            out[n, :, pa * 16:pa * 16 + 16, 48 * hh:48 * hh + 48] = img[pa]
    return out
